# revision 14
# baseline (speedup 1.0000x reference)
"""NonNegLinear forward (eval path) on 8 Trainium2 NeuronCores.

reference:
    w = clip(weight, 0, 5)                       # [C, P]
    importance[b, p, c] = input[b, p] * w[c, p]  # [B, P, C]  (>= 0, threshold no-op)
    logits = importance.sum(axis=1) + bias       # [B, C]

Shapes: B=128, P=1024, C=1000, fp32.

Strategy: data-parallel over batch — each of the 8 cores handles 16 batch
rows and the full weight.  Per core the dominant cost is writing its
16*1024*1000*4 = 65.5 MB importance shard, so the kernel is laid out around
streaming those writes at HBM rate:

  - weight is loaded [c, p] (natural layout), transposed 128x128-blockwise on
    the TensorEngine into wT [p, c] tiles, with clip(0,5) fused into the
    PSUM->SBUF copy.
  - input shard is transposed the same way into inputT [p, b] columns.
  - importance tile [128p x 1000c] = wT_tile * input[b, p_tile] broadcast
    (per-partition scalar) on the Vector/Scalar engines, then one 4 MB DMA
    per batch row writes the [1024, 1000] block contiguously.
  - logits = inputT.T @ wT accumulated on the TensorEngine over the 8
    p-tiles, bias added as an extra K=1 matmul into the same PSUM tile.
"""

import os
import sys

import numpy as np

if "/opt/trn_rl_repo" not in sys.path:
    sys.path.insert(0, "/opt/trn_rl_repo")

import concourse.bass as bass
import concourse.mybir as mybir
import concourse.tile as tile
from concourse import bacc, masks
from concourse.bass_utils import run_bass_kernel_spmd

N_CORES = 8
B_FULL, P, C = 128, 1024, 1000
B = B_FULL // N_CORES  # 16 batch rows per core
PT = P // 128  # 8 p-tiles
F32 = mybir.dt.float32

LAST_RESULTS = None  # BassKernelResults of the most recent run (for test.py)


def _build_program(b_rows=B):
    nc = bacc.Bacc()

    inp = nc.dram_tensor("input", [B, P], F32, kind="ExternalInput")
    weight = nc.dram_tensor("weight", [C, P], F32, kind="ExternalInput")
    bias_d = nc.dram_tensor("bias", [C], F32, kind="ExternalInput")
    imp = nc.dram_tensor("importance", [B, P, C], F32, kind="ExternalOutput")
    logits_d = nc.dram_tensor("logits", [B, C], F32, kind="ExternalOutput")

    CF = (C // 128) * 128  # 896: full 128-row chunks of C
    CR = C - CF  # 104: ragged remainder

    with tile.TileContext(nc) as tc:
        with (
            tc.tile_pool(name="const", bufs=1) as const_pool,
            tc.tile_pool(name="wt", bufs=1) as wt_pool,
            tc.tile_pool(name="wstage", bufs=3) as wstage_pool,
            tc.tile_pool(name="out", bufs=6) as out_pool,
            tc.tile_pool(name="psum_t", bufs=4, space="PSUM") as psum_t_pool,
            tc.tile_pool(name="psum_l", bufs=2, space="PSUM") as psum_l_pool,
        ):
            identity = const_pool.tile([128, 128], F32, tag="identity")
            masks.make_identity(nc, identity[:])

            input_sb = const_pool.tile([B, P], F32, tag="input_sb")
            bias_sb = const_pool.tile([1, C], F32, tag="bias_sb")
            ones = const_pool.tile([1, B], F32, tag="ones")
            nc.vector.memset(ones[:], 1.0)
            inputT = const_pool.tile([128, PT * B], F32, tag="inputT")
            nc.scalar.dma_start(input_sb[:], inp[:])
            nc.scalar.dma_start(bias_sb[:], bias_d[None, :])
            # inputT[p_i, pj*B + b] = input[b, pj*128 + p_i]
            for pj in range(PT):
                pt = psum_t_pool.tile([128, 128], F32, tag="tp")
                nc.tensor.transpose(
                    pt[:, :B],
                    input_sb[:, pj * 128 : (pj + 1) * 128],
                    identity[:B, :B],
                )
                nc.vector.tensor_copy(inputT[:, pj * B : (pj + 1) * B], pt[:, :B])

            # wT tiles: wt[pi][p_i, c] = clip(weight[c, pi*128 + p_i], 0, 5).
            # Loaded column-chunk-wise (one fused strided DMA per p-tile plus
            # the ragged tail) so each wT tile completes as early as possible
            # and the importance stores for that p-tile can start streaming.
            wt = [
                wt_pool.tile([128, C], F32, tag=f"wt{pi}", name=f"wt{pi}")
                for pi in range(PT)
            ]
            for pi in range(PT):
                wstage = wstage_pool.tile(
                    [128, P], F32, tag="wstage", name=f"wstage{pi}"
                )
                # wstage[c_i, ci*128 + p] = weight[ci*128 + c_i, pi*128 + p]
                nc.scalar.dma_start(
                    wstage[:, :CF].rearrange("c (ci p) -> c ci p", p=128),
                    weight[:CF, pi * 128 : (pi + 1) * 128].rearrange(
                        "(ci c) p -> c ci p", c=128
                    ),
                )
                nc.scalar.dma_start(
                    wstage[:CR, CF:], weight[CF:, pi * 128 : (pi + 1) * 128]
                )
                for ci in range((C + 127) // 128):
                    cn = min(128, C - ci * 128)
                    pt = psum_t_pool.tile([128, 128], F32, tag="tp")
                    nc.tensor.transpose(
                        pt[:, :cn],
                        wstage[:cn, ci * 128 : (ci + 1) * 128],
                        identity[:cn, :cn],
                    )
                    # fused clip(x, 0, 5) on the PSUM->SBUF copy
                    nc.vector.tensor_scalar(
                        wt[pi][:, ci * 128 : ci * 128 + cn],
                        pt[:, :cn],
                        5.0,
                        0.0,
                        op0=mybir.AluOpType.min,
                        op1=mybir.AluOpType.max,
                    )

                # importance stores for this p-tile: [128, C] -> 500 KB DMA
                for b in range(b_rows):
                    out_t = out_pool.tile([128, C], F32, tag="out")
                    scal = inputT[:, pi * B + b : pi * B + b + 1]
                    if (pi * b_rows + b) % 3 == 2:
                        nc.scalar.mul(out_t[:], wt[pi][:], scal)
                    else:
                        nc.vector.tensor_scalar_mul(out_t[:], wt[pi][:], scal)
                    nc.sync.dma_start(
                        imp[b, pi * 128 : (pi + 1) * 128, :], out_t[:]
                    )

            # logits = inputT.T @ wT + bias, accumulated in PSUM.  Emitted
            # before the store stream so it rides in the gaps.
            logits_sb = const_pool.tile([B, C], F32, tag="logits_sb")
            for c0 in range(0, C, 512):
                cw = min(512, C - c0)
                pl = psum_l_pool.tile([B, 512], F32, tag="pl")
                for pi in range(PT):
                    nc.tensor.matmul(
                        pl[:, :cw],
                        lhsT=inputT[:, pi * B : (pi + 1) * B],
                        rhs=wt[pi][:, c0 : c0 + cw],
                        start=(pi == 0),
                        stop=False,
                    )
                nc.tensor.matmul(
                    pl[:, :cw],
                    lhsT=ones[:, :],
                    rhs=bias_sb[:, c0 : c0 + cw],
                    start=False,
                    stop=True,
                )
                nc.scalar.copy(logits_sb[:, c0 : c0 + cw], pl[:, :cw])
            nc.scalar.dma_start(logits_d[:], logits_sb[:])

    nc.compile()
    return nc


_PROGRAM = None


def kernel(input, weight, bias):
    global _PROGRAM, LAST_RESULTS

    input = np.ascontiguousarray(input, dtype=np.float32)
    weight = np.ascontiguousarray(weight, dtype=np.float32)
    bias = np.ascontiguousarray(bias, dtype=np.float32)
    assert input.shape == (B_FULL, P) and weight.shape == (C, P)

    if _PROGRAM is None:
        _PROGRAM = _build_program()

    in_maps = [
        {
            "input": input[i * B : (i + 1) * B],
            "weight": weight,
            "bias": bias,
        }
        for i in range(N_CORES)
    ]
    res = run_bass_kernel_spmd(
        _PROGRAM,
        in_maps,
        core_ids=list(range(N_CORES)),
        trace=bool(int(os.environ.get("KERNEL_TRACE", "0"))),
    )
    LAST_RESULTS = res

    importance = np.concatenate([r["importance"] for r in res.results], axis=0)
    logits = np.concatenate([r["logits"] for r in res.results], axis=0)
    return importance, logits


# revision 18
# speedup vs baseline: 162548.3179x; 162548.3179x over previous
"""NonNegLinear forward (eval path) on 8 Trainium2 NeuronCores.

reference:
    w = clip(weight, 0, 5)                       # [C, P]
    importance[b, p, c] = input[b, p] * w[c, p]  # [B, P, C]  (>= 0, threshold no-op)
    logits = importance.sum(axis=1) + bias       # [B, C]

Shapes: B=128, P=1024, C=1000, fp32.

Strategy: data-parallel over batch — each of the 8 cores handles 16 batch
rows and the full weight.  Per core the dominant cost is writing its
16*1024*1000*4 = 65.5 MB importance shard, so the kernel is laid out around
streaming those writes at HBM rate:

  - weight is loaded column-chunk-wise ([1000, 128] slabs, 512 B descriptor
    runs = still full DMA rate) so each p-tile of the transposed weight
    completes as early as possible; each slab is transposed 128x128-blockwise
    on the TensorEngine with clip(0,5) fused into the PSUM->SBUF copy.
  - input shard is transposed the same way into inputT [p, b] columns.
  - importance tile [128p x 1000c] = wT_tile * input[b, p_tile] broadcast
    (per-partition scalar; fp32 2x mode) on the Vector/Scalar engines, then a
    500 KB contiguous DMA per (p-tile, row) streams it out.  Stores ride the
    SP HWDGE ring, loads the ACT ring, so neither stalls the other.
  - logits = inputT.T @ wT accumulated on the TensorEngine over the 8
    p-tiles, bias added as an extra K=1 matmul into the same PSUM tile.

Cost-model timeline: ~199.5 us/core, DMA engines 97% occupied (serialized
DMA byte time is 193.8 us; the roofline for the 69.8 MB/core of traffic).
"""

import os
import sys

import numpy as np

if "/opt/trn_rl_repo" not in sys.path:
    sys.path.insert(0, "/opt/trn_rl_repo")

import concourse.mybir as mybir
import concourse.tile as tile
from concourse import bacc, masks
from concourse.bass_utils import run_bass_kernel_spmd

N_CORES = 8
B_FULL, P, C = 128, 1024, 1000
B = B_FULL // N_CORES  # 16 batch rows per core
PT = P // 128  # 8 p-tiles
F32 = mybir.dt.float32

LAST_RESULTS = None  # BassKernelResults of the most recent run (for test.py)


def _build_program(b_rows=B):
    nc = bacc.Bacc()

    inp = nc.dram_tensor("input", [B, P], F32, kind="ExternalInput")
    weight = nc.dram_tensor("weight", [C, P], F32, kind="ExternalInput")
    bias_d = nc.dram_tensor("bias", [C], F32, kind="ExternalInput")
    imp = nc.dram_tensor("importance", [B, P, C], F32, kind="ExternalOutput")
    logits_d = nc.dram_tensor("logits", [B, C], F32, kind="ExternalOutput")

    CF = (C // 128) * 128  # 896: full 128-row chunks of C
    CR = C - CF  # 104: ragged remainder

    with tile.TileContext(nc) as tc:
        with (
            tc.tile_pool(name="const", bufs=1) as const_pool,
            tc.tile_pool(name="wt", bufs=1) as wt_pool,
            tc.tile_pool(name="wstage", bufs=6) as wstage_pool,
            tc.tile_pool(name="out", bufs=6) as out_pool,
            tc.tile_pool(name="psum_t", bufs=4, space="PSUM") as psum_t_pool,
            tc.tile_pool(name="psum_l", bufs=2, space="PSUM") as psum_l_pool,
        ):
            identity = const_pool.tile([128, 128], F32, tag="identity")
            masks.make_identity(nc, identity[:])

            input_sb = const_pool.tile([B, P], F32, tag="input_sb")
            bias_sb = const_pool.tile([1, C], F32, tag="bias_sb")
            ones = const_pool.tile([1, B], F32, tag="ones")
            nc.vector.memset(ones[:], 1.0)
            inputT = const_pool.tile([128, PT * B], F32, tag="inputT")
            nc.scalar.dma_start(input_sb[:], inp[:])
            nc.scalar.dma_start(bias_sb[:], bias_d[None, :])
            # inputT[p_i, pj*B + b] = input[b, pj*128 + p_i]
            for pj in range(PT):
                pt = psum_t_pool.tile([128, 128], F32, tag="tp")
                nc.tensor.transpose(
                    pt[:, :B],
                    input_sb[:, pj * 128 : (pj + 1) * 128],
                    identity[:B, :B],
                )
                nc.vector.tensor_copy(inputT[:, pj * B : (pj + 1) * B], pt[:, :B])

            # wT tiles: wt[pi][p_i, c] = clip(weight[c, pi*128 + p_i], 0, 5).
            # Loaded column-chunk-wise (one fused strided DMA per p-tile plus
            # the ragged tail) so each wT tile completes as early as possible
            # and the importance stores for that p-tile can start streaming.
            wt = [
                wt_pool.tile([128, C], F32, tag=f"wt{pi}", name=f"wt{pi}")
                for pi in range(PT)
            ]
            for pi in range(PT):
                wstage = wstage_pool.tile(
                    [128, P], F32, tag="wstage", name=f"wstage{pi}"
                )
                # wstage[c_i, ci*128 + p] = weight[ci*128 + c_i, pi*128 + p]
                nc.scalar.dma_start(
                    wstage[:, :CF].rearrange("c (ci p) -> c ci p", p=128),
                    weight[:CF, pi * 128 : (pi + 1) * 128].rearrange(
                        "(ci c) p -> c ci p", c=128
                    ),
                )
                nc.scalar.dma_start(
                    wstage[:CR, CF:], weight[CF:, pi * 128 : (pi + 1) * 128]
                )
                for ci in range((C + 127) // 128):
                    cn = min(128, C - ci * 128)
                    pt = psum_t_pool.tile([128, 128], F32, tag="tp")
                    nc.tensor.transpose(
                        pt[:, :cn],
                        wstage[:cn, ci * 128 : (ci + 1) * 128],
                        identity[:cn, :cn],
                    )
                    # fused clip(x, 0, 5) on the PSUM->SBUF copy
                    nc.vector.tensor_scalar(
                        wt[pi][:, ci * 128 : ci * 128 + cn],
                        pt[:, :cn],
                        5.0,
                        0.0,
                        op0=mybir.AluOpType.min,
                        op1=mybir.AluOpType.max,
                    )

                # importance stores for this p-tile: [128, C] -> 500 KB DMA
                for b in range(b_rows):
                    out_t = out_pool.tile([128, C], F32, tag="out")
                    scal = inputT[:, pi * B + b : pi * B + b + 1]
                    if (pi * b_rows + b) % 3 == 2:
                        nc.scalar.mul(out_t[:], wt[pi][:], scal)
                    else:
                        nc.vector.tensor_scalar_mul(out_t[:], wt[pi][:], scal)
                    nc.sync.dma_start(
                        imp[b, pi * 128 : (pi + 1) * 128, :], out_t[:]
                    )

            # logits = inputT.T @ wT + bias, accumulated in PSUM.  Emitted
            # before the store stream so it rides in the gaps.
            logits_sb = const_pool.tile([B, C], F32, tag="logits_sb")
            for c0 in range(0, C, 512):
                cw = min(512, C - c0)
                pl = psum_l_pool.tile([B, 512], F32, tag="pl")
                for pi in range(PT):
                    nc.tensor.matmul(
                        pl[:, :cw],
                        lhsT=inputT[:, pi * B : (pi + 1) * B],
                        rhs=wt[pi][:, c0 : c0 + cw],
                        start=(pi == 0),
                        stop=False,
                    )
                nc.tensor.matmul(
                    pl[:, :cw],
                    lhsT=ones[:, :],
                    rhs=bias_sb[:, c0 : c0 + cw],
                    start=False,
                    stop=True,
                )
                nc.scalar.copy(logits_sb[:, c0 : c0 + cw], pl[:, :cw])
            nc.scalar.dma_start(logits_d[:], logits_sb[:])

    nc.compile()
    return nc


_PROGRAM = None


def kernel(input, weight, bias):
    global _PROGRAM, LAST_RESULTS

    input = np.ascontiguousarray(input, dtype=np.float32)
    weight = np.ascontiguousarray(weight, dtype=np.float32)
    bias = np.ascontiguousarray(bias, dtype=np.float32)
    assert input.shape == (B_FULL, P) and weight.shape == (C, P)

    if _PROGRAM is None:
        _PROGRAM = _build_program()

    in_maps = [
        {
            "input": input[i * B : (i + 1) * B],
            "weight": weight,
            "bias": bias,
        }
        for i in range(N_CORES)
    ]
    res = run_bass_kernel_spmd(
        _PROGRAM,
        in_maps,
        core_ids=list(range(N_CORES)),
        trace=bool(int(os.environ.get("KERNEL_TRACE", "0"))),
    )
    LAST_RESULTS = res

    importance = np.concatenate([r["importance"] for r in res.results], axis=0)
    logits = np.concatenate([r["logits"] for r in res.results], axis=0)
    return importance, logits



# revision 28
# speedup vs baseline: 164269.0245x; 1.0106x over previous
"""NonNegLinear forward (eval path) on 8 Trainium2 NeuronCores.

reference:
    w = clip(weight, 0, 5)                       # [C, P]
    importance[b, p, c] = input[b, p] * w[c, p]  # [B, P, C]  (>= 0, threshold no-op)
    logits = importance.sum(axis=1) + bias       # [B, C]

Shapes: B=128, P=1024, C=1000, fp32.

Strategy: data-parallel over batch — each of the 8 cores handles 16 batch
rows and the full weight.  Per core the dominant cost is writing its
16*1024*1000*4 = 65.5 MB importance shard, so the kernel is laid out around
streaming those writes at HBM rate:

  - weight is loaded column-chunk-wise ([1000, 128] slabs, 512 B descriptor
    runs = still full DMA rate) so each p-tile of the transposed weight
    completes as early as possible; each slab is transposed 128x128-blockwise
    on the TensorEngine with clip(0,5) fused into the PSUM->SBUF copy.
  - input shard is transposed the same way into inputT [p, b] columns.
  - importance tile [128p x 1000c] = wT_tile * input[b, p_tile] broadcast
    (per-partition scalar; fp32 2x mode) on the Vector/Scalar engines, then a
    500 KB contiguous DMA per (p-tile, row) streams it out.  Stores ride the
    SP HWDGE ring and weight slabs the ACT ring (slab 1 on SP so the two
    rings' DGE delays overlap at startup); the tiny input/bias loads go via
    SWDGE (gpsimd) to keep their issue holds off the shared HWDGE device.
  - logits = inputT.T @ wT accumulated on the TensorEngine over the 8
    p-tiles, bias added as an extra K=1 matmul into the same PSUM tile.

Cost-model timeline: 197.4 us/core with a gap-free DMA stream (serialized
DMA byte time is 193.8 us; the roofline for the 69.8 MB/core of traffic).
"""

import os
import sys

import numpy as np

if "/opt/trn_rl_repo" not in sys.path:
    sys.path.insert(0, "/opt/trn_rl_repo")

import concourse.mybir as mybir
import concourse.tile as tile
from concourse import bacc, masks
from concourse.bass_utils import run_bass_kernel_spmd

N_CORES = 8
B_FULL, P, C = 128, 1024, 1000
B = B_FULL // N_CORES  # 16 batch rows per core
PT = P // 128  # 8 p-tiles
F32 = mybir.dt.float32

LAST_RESULTS = None  # BassKernelResults of the most recent run (for test.py)


def _build_program(b_rows=B):
    nc = bacc.Bacc()

    inp = nc.dram_tensor("input", [B, P], F32, kind="ExternalInput")
    weight = nc.dram_tensor("weight", [C, P], F32, kind="ExternalInput")
    bias_d = nc.dram_tensor("bias", [C], F32, kind="ExternalInput")
    imp = nc.dram_tensor("importance", [B, P, C], F32, kind="ExternalOutput")
    logits_d = nc.dram_tensor("logits", [B, C], F32, kind="ExternalOutput")

    CF = (C // 128) * 128  # 896: full 128-row chunks of C
    CR = C - CF  # 104: ragged remainder

    with tile.TileContext(nc) as tc:
        with (
            tc.tile_pool(name="const", bufs=1) as const_pool,
            tc.tile_pool(name="wt", bufs=1) as wt_pool,
            tc.tile_pool(name="wstage", bufs=6) as wstage_pool,
            tc.tile_pool(name="out", bufs=6) as out_pool,
            tc.tile_pool(name="psum_t", bufs=4, space="PSUM") as psum_t_pool,
            tc.tile_pool(name="psum_l", bufs=2, space="PSUM") as psum_l_pool,
        ):
            identity = const_pool.tile([128, 128], F32, tag="identity")
            masks.make_identity(nc, identity[:])

            input_sb = const_pool.tile([B, P], F32, tag="input_sb")
            bias_sb = const_pool.tile([1, C], F32, tag="bias_sb")
            ones = const_pool.tile([1, B], F32, tag="ones")
            nc.vector.memset(ones[:], 1.0)
            inputT = const_pool.tile([128, PT * B], F32, tag="inputT")

            # wT tiles: wt[pi][p_i, c] = clip(weight[c, pi*128 + p_i], 0, 5).
            # Loaded column-chunk-wise (one fused strided DMA per p-tile plus
            # the ragged tail) so each wT tile completes as early as possible
            # and the importance stores for that p-tile can start streaming.
            wt = [
                wt_pool.tile([128, C], F32, tag=f"wt{pi}", name=f"wt{pi}")
                for pi in range(PT)
            ]
            for pi in range(PT):
                wstage = wstage_pool.tile(
                    [128, P], F32, tag="wstage", name=f"wstage{pi}"
                )
                # wstage[c_i, ci*128 + p] = weight[ci*128 + c_i, pi*128 + p]
                ldeng = nc.sync if pi == 1 else nc.scalar
                ldeng.dma_start(
                    wstage[:, :CF].rearrange("c (ci p) -> c ci p", p=128),
                    weight[:CF, pi * 128 : (pi + 1) * 128].rearrange(
                        "(ci c) p -> c ci p", c=128
                    ),
                )
                ldeng.dma_start(
                    wstage[:CR, CF:], weight[CF:, pi * 128 : (pi + 1) * 128]
                )
                if pi == 0:
                    nc.gpsimd.dma_start(input_sb[:], inp[:])
                    # inputT[p_i, pj*B + b] = input[b, pj*128 + p_i]
                    for pj in range(PT):
                        pt = psum_t_pool.tile([128, 128], F32, tag="tp")
                        nc.tensor.transpose(
                            pt[:, :B],
                            input_sb[:, pj * 128 : (pj + 1) * 128],
                            identity[:B, :B],
                        )
                        nc.vector.tensor_copy(
                            inputT[:, pj * B : (pj + 1) * B], pt[:, :B]
                        )
                for ci in range((C + 127) // 128):
                    cn = min(128, C - ci * 128)
                    pt = psum_t_pool.tile([128, 128], F32, tag="tp")
                    nc.tensor.transpose(
                        pt[:, :cn],
                        wstage[:cn, ci * 128 : (ci + 1) * 128],
                        identity[:cn, :cn],
                    )
                    # fused clip(x, 0, 5) on the PSUM->SBUF copy
                    nc.vector.tensor_scalar(
                        wt[pi][:, ci * 128 : ci * 128 + cn],
                        pt[:, :cn],
                        5.0,
                        0.0,
                        op0=mybir.AluOpType.min,
                        op1=mybir.AluOpType.max,
                    )

                # importance stores for this p-tile: [128, C] -> 500 KB DMA
                for b in range(b_rows):
                    out_t = out_pool.tile([128, C], F32, tag="out")
                    scal = inputT[:, pi * B + b : pi * B + b + 1]
                    if (pi * b_rows + b) % 3 == 2:
                        nc.scalar.mul(out_t[:], wt[pi][:], scal)
                    else:
                        nc.vector.tensor_scalar_mul(out_t[:], wt[pi][:], scal)
                    nc.sync.dma_start(
                        imp[b, pi * 128 : (pi + 1) * 128, :], out_t[:]
                    )

            # logits = inputT.T @ wT + bias, accumulated in PSUM.  Emitted
            # before the store stream so it rides in the gaps.
            logits_sb = const_pool.tile([B, C], F32, tag="logits_sb")
            nc.gpsimd.dma_start(bias_sb[:], bias_d[None, :])
            for c0 in range(0, C, 512):
                cw = min(512, C - c0)
                pl = psum_l_pool.tile([B, 512], F32, tag="pl")
                for pi in range(PT):
                    nc.tensor.matmul(
                        pl[:, :cw],
                        lhsT=inputT[:, pi * B : (pi + 1) * B],
                        rhs=wt[pi][:, c0 : c0 + cw],
                        start=(pi == 0),
                        stop=False,
                    )
                nc.tensor.matmul(
                    pl[:, :cw],
                    lhsT=ones[:, :],
                    rhs=bias_sb[:, c0 : c0 + cw],
                    start=False,
                    stop=True,
                )
                nc.scalar.copy(logits_sb[:, c0 : c0 + cw], pl[:, :cw])
            nc.scalar.dma_start(logits_d[:], logits_sb[:])

    nc.compile()
    return nc


_PROGRAM = None


def kernel(input, weight, bias):
    global _PROGRAM, LAST_RESULTS

    input = np.ascontiguousarray(input, dtype=np.float32)
    weight = np.ascontiguousarray(weight, dtype=np.float32)
    bias = np.ascontiguousarray(bias, dtype=np.float32)
    assert input.shape == (B_FULL, P) and weight.shape == (C, P)

    if _PROGRAM is None:
        _PROGRAM = _build_program()

    in_maps = [
        {
            "input": input[i * B : (i + 1) * B],
            "weight": weight,
            "bias": bias,
        }
        for i in range(N_CORES)
    ]
    res = run_bass_kernel_spmd(
        _PROGRAM,
        in_maps,
        core_ids=list(range(N_CORES)),
        trace=bool(int(os.environ.get("KERNEL_TRACE", "0"))),
    )
    LAST_RESULTS = res

    importance = np.concatenate([r["importance"] for r in res.results], axis=0)
    logits = np.concatenate([r["logits"] for r in res.results], axis=0)
    return importance, logits

